# revision 6
# baseline (speedup 1.0000x reference)
"""Trainium2 Bass kernel for nn_Actor (sampling): per-row Gaussian-grid
softmax + per-row gather, data-parallel over 32768 distribution rows on 8
NeuronCores.

Math: for each row r (r = lut*512 + j), mean m_r = tanh(x @ W + b)[r].
With grid g_i = -1 + 2i/127, std 0.5:
    probs[r, i] = exp(-0.5*((g_i - m_r)/0.5)^2 + c0),  c0 = -ln(0.5) - 0.5*ln(2pi)
    dist[r, i]  = exp(probs[r, i]) / S_r,   S_r = sum_i exp(probs[r, i])
    alp[e, r]   = dist[r, action[e, r]]

Key simplifications:
  * The gather is recomputed directly from the action index a:
    z = s2*a + b_r (s2 = (2/127)/0.5, b_r = -2 - 2*m_r), so
    alp = exp(exp(-0.5*z^2 + c0)) * invS_r  -- pure elementwise work.
  * S_r is an analytic, even function of m_r alone; 1/S is evaluated by a
    hard-coded degree-8 polynomial in u = m^2 (max rel err ~1.6e-9), so the
    128-point grid sweep is never materialized.

Per core: rows [c*4096, (c+1)*4096), actions arrive host-transposed as
bf16 [4096, 128] (exact small ints), outputs leave as [4096, 128] f32 and
are transposed back on the host.
"""

import os
import math
import numpy as np
import ml_dtypes

N_CORES = 8
ROWS = 32768
R = ROWS // N_CORES          # 4096 rows per core
E = 128                      # epochs
G = R // 128                 # 32 row-groups of 128
FEAT = 256
M_OUT = 512
LUTS_PER_CORE = 8
X_DIM = 128
THR_STD = 0.5
X_RANGE = 1.0

S2 = float((2.0 * X_RANGE / (X_DIM - 1)) / THR_STD)
C0 = float(-math.log(THR_STD) - 0.5 * math.log(2.0 * math.pi))

# 1/S(m) as an even polynomial in u = m^2 (degree 8, rel err ~1.6e-9),
# S(m) = sum_i exp(exp(-0.5*((g_i - m)/std)^2 + c0)) over the fixed grid.
INVS_COEF = [
    0.004734368204497658,
    0.0006664708139734746,
    0.0005536234448627335,
    -3.841141713816861e-05,
    -8.543615513324016e-05,
    -3.230374091879953e-05,
    3.099837949298949e-05,
    -2.056325921731732e-06,
    -1.460328775896884e-06,
]

CH = 16                      # row-groups per compute chunk ([128, CH*128] tiles)
SQ_ON_DVE = 0                # chunks (of 4) whose Square runs on VectorE

_CACHE = {}


def _col(d, g):
    # column of the per-row vectors (m, b, invS) holding group g of dist d.
    # Group g covers rows [g*128, (g+1)*128); row r = i*512 + jc*128 + p
    # (i = lut, jc = j-chunk, p = partition) => g = i*4 + jc.
    i, jc = g // 4, g % 4
    return d * 32 + jc * 8 + i


def _build():
    if "nc" in _CACHE:
        return _CACHE["nc"]

    from contextlib import ExitStack
    import concourse.bacc as bacc
    import concourse.mybir as mybir
    from concourse.tile import TileContext
    from concourse.alu_op_type import AluOpType

    dt = mybir.dt
    AF = mybir.ActivationFunctionType

    nc = bacc.Bacc("TRN2", target_bir_lowering=False, debug=False)

    a_ext = [
        nc.declare_dram_parameter("ax_t", [R, E], dt.bfloat16, isOutput=False),
        nc.declare_dram_parameter("ay_t", [R, E], dt.bfloat16, isOutput=False),
    ]
    w_ext = [
        nc.declare_dram_parameter("w_x", [FEAT, M_OUT], dt.float32, isOutput=False),
        nc.declare_dram_parameter("w_y", [FEAT, M_OUT], dt.float32, isOutput=False),
    ]
    b_ext = [
        nc.declare_dram_parameter("b_x", [1, M_OUT], dt.float32, isOutput=False),
        nc.declare_dram_parameter("b_y", [1, M_OUT], dt.float32, isOutput=False),
    ]
    xt_ext = nc.declare_dram_parameter(
        "x_t", [FEAT, LUTS_PER_CORE], dt.float32, isOutput=False
    )
    o_ext = [
        nc.declare_dram_parameter("out_x", [R, E], dt.float32, isOutput=True),
        nc.declare_dram_parameter("out_y", [R, E], dt.float32, isOutput=True),
    ]

    with TileContext(nc) as tc, ExitStack() as ctx:
        const = ctx.enter_context(tc.tile_pool(name="const", bufs=1))
        psum = ctx.enter_context(tc.tile_pool(name="psum", bufs=2, space="PSUM"))
        zpool = ctx.enter_context(tc.tile_pool(name="zpool", bufs=2))
        z2pool = ctx.enter_context(tc.tile_pool(name="z2pool", bufs=2))
        ppool = ctx.enter_context(tc.tile_pool(name="ppool", bufs=2))
        epool = ctx.enter_context(tc.tile_pool(name="epool", bufs=2))
        opool = ctx.enter_context(tc.tile_pool(name="opool", bufs=3))

        # ---- constant / parameter loads -------------------------------
        w_sb = []
        b_sb = []
        a_sb = []
        for d in range(2):
            w = const.tile([128, 2, M_OUT], dt.float32, tag=f"w{d}")
            nc.sync.dma_start(
                out=w, in_=w_ext[d].ap().rearrange("(kc p) j -> p kc j", p=128)
            )
            w_sb.append(w)
            b = const.tile([1, M_OUT], dt.float32, tag=f"b{d}")
            nc.sync.dma_start(out=b, in_=b_ext[d].ap())
            b_sb.append(b)
            a = const.tile([128, G, E], dt.bfloat16, tag=f"a{d}")
            nc.sync.dma_start(
                out=a, in_=a_ext[d].ap().rearrange("(g p) e -> p g e", p=128)
            )
            a_sb.append(a)
        xt_sb = const.tile([128, 2, LUTS_PER_CORE], dt.float32, name="xt")
        nc.sync.dma_start(
            out=xt_sb, in_=xt_ext.ap().rearrange("(kc p) i -> p kc i", p=128)
        )
        ones = const.tile([1, LUTS_PER_CORE], dt.float32, name="ones")
        nc.vector.memset(ones, 1.0)
        c0_bias = const.tile([128, 1], dt.float32, name="c0_bias")
        nc.vector.memset(c0_bias, C0)

        # ---- means: meanT = tanh((x @ W + b)^T), computed transposed ---
        # psum[j, i] = sum_k W[k, j] * x[i, k]  (+ b[j] via a K=1 matmul)
        m_all = const.tile([128, 64], dt.float32, name="m_all")
        for d in range(2):
            pm = psum.tile([128, 32], dt.float32)
            for jc in range(4):
                o = pm[:, jc * 8:(jc + 1) * 8]
                nc.tensor.matmul(
                    o,
                    lhsT=w_sb[d][:, 0, jc * 128:(jc + 1) * 128],
                    rhs=xt_sb[:, 0, :],
                    start=True, stop=False,
                )
                nc.tensor.matmul(
                    o,
                    lhsT=w_sb[d][:, 1, jc * 128:(jc + 1) * 128],
                    rhs=xt_sb[:, 1, :],
                    start=False, stop=False,
                )
                nc.tensor.matmul(
                    o,
                    lhsT=b_sb[d][0:1, jc * 128:(jc + 1) * 128],
                    rhs=ones[0:1, :],
                    start=False, stop=True,
                )
            nc.scalar.activation(m_all[:, d * 32:(d + 1) * 32], pm, AF.Tanh)

        # ---- per-row prep: b_r = -2 - 2m, invS = poly(m^2) -------------
        u = const.tile([128, 64], dt.float32, name="u")
        nc.vector.tensor_tensor(u, m_all, m_all, AluOpType.mult)
        b_all = const.tile([128, 64], dt.float32, name="b_all")
        nc.vector.tensor_scalar(
            b_all, m_all, -2.0, -2.0, AluOpType.mult, AluOpType.add
        )
        h = const.tile([128, 64], dt.float32, name="h")
        t = const.tile([128, 64], dt.float32, name="t")
        nc.vector.tensor_scalar(
            h, u, float(INVS_COEF[8]), float(INVS_COEF[7]),
            AluOpType.mult, AluOpType.add,
        )
        for k in range(6, -1, -1):
            nc.vector.tensor_tensor(t, h, u, AluOpType.mult)
            nc.vector.tensor_scalar(h, t, float(INVS_COEF[k]), None, AluOpType.add)
        invs_all = h

        # ---- main loop: alp = exp(exp(-0.5 z^2 + c0)) * invS -----------
        n_chunks = G // CH
        chunk_idx = 0
        for d in range(2):
            o_re = o_ext[d].ap().rearrange("(g p) e -> p g e", p=128)
            for c2 in range(n_chunks):
                z = zpool.tile([128, CH, E], dt.float32)
                for gi in range(CH):
                    g = c2 * CH + gi
                    col = _col(d, g)
                    nc.vector.tensor_scalar(
                        z[:, gi, :], a_sb[d][:, g, :],
                        S2, b_all[:, col:col + 1],
                        AluOpType.mult, AluOpType.add,
                    )
                z2 = z2pool.tile([128, CH, E], dt.float32)
                if chunk_idx < SQ_ON_DVE:
                    for gi in range(CH):
                        nc.vector.tensor_tensor(
                            z2[:, gi, :], z[:, gi, :], z[:, gi, :], AluOpType.mult
                        )
                else:
                    nc.scalar.activation(z2, z, AF.Square)
                p = ppool.tile([128, CH, E], dt.float32)
                nc.scalar.activation(p, z2, AF.Exp, bias=c0_bias, scale=-0.5)
                e = epool.tile([128, CH, E], dt.float32)
                nc.scalar.activation(e, p, AF.Exp)
                o = opool.tile([128, CH, E], dt.float32)
                for gi in range(CH):
                    g = c2 * CH + gi
                    col = _col(d, g)
                    nc.vector.tensor_scalar(
                        o[:, gi, :], e[:, gi, :],
                        invs_all[:, col:col + 1], None,
                        AluOpType.mult,
                    )
                nc.sync.dma_start(
                    out=o_re[:, c2 * CH:(c2 + 1) * CH, :], in_=o
                )
                chunk_idx += 1

    nc.compile()
    _CACHE["nc"] = nc
    return nc


LAST_RESULTS = None


def kernel(x, Wx, bx, Wy, by, action_x, action_y):
    global LAST_RESULTS
    from concourse.bass_utils import run_bass_kernel_spmd

    nc = _build()

    x = np.ascontiguousarray(np.asarray(x, dtype=np.float32))
    Wx = np.ascontiguousarray(np.asarray(Wx, dtype=np.float32))
    Wy = np.ascontiguousarray(np.asarray(Wy, dtype=np.float32))
    bx = np.ascontiguousarray(np.asarray(bx, dtype=np.float32)).reshape(1, M_OUT)
    by = np.ascontiguousarray(np.asarray(by, dtype=np.float32)).reshape(1, M_OUT)
    # action values are small ints (0..127): exactly representable in bf16
    ax_t = np.asarray(action_x).T.astype(np.float32).astype(ml_dtypes.bfloat16)
    ay_t = np.asarray(action_y).T.astype(np.float32).astype(ml_dtypes.bfloat16)

    in_maps = []
    for c in range(N_CORES):
        in_maps.append({
            "ax_t": np.ascontiguousarray(ax_t[c * R:(c + 1) * R]),
            "ay_t": np.ascontiguousarray(ay_t[c * R:(c + 1) * R]),
            "w_x": Wx,
            "w_y": Wy,
            "b_x": bx,
            "b_y": by,
            "x_t": np.ascontiguousarray(
                x[c * LUTS_PER_CORE:(c + 1) * LUTS_PER_CORE].T
            ),
        })

    res = run_bass_kernel_spmd(nc, in_maps, core_ids=list(range(N_CORES)))
    LAST_RESULTS = res

    alp_x = np.empty((E, ROWS), dtype=np.float32)
    alp_y = np.empty((E, ROWS), dtype=np.float32)
    for c in range(N_CORES):
        alp_x[:, c * R:(c + 1) * R] = res.results[c]["out_x"].T
        alp_y[:, c * R:(c + 1) * R] = res.results[c]["out_y"].T
    ent = np.float32(0.5 + 0.5 * math.log(2.0 * math.pi) + math.log(THR_STD))
    return alp_x, alp_y, ent, ent


# revision 7
# speedup vs baseline: 1.1842x; 1.1842x over previous
"""Trainium2 Bass kernel for nn_Actor (sampling): per-row Gaussian-grid
softmax + per-row gather, data-parallel over 32768 distribution rows on 8
NeuronCores.

Math: for each row r (r = lut*512 + j), mean m_r = tanh(x @ W + b)[r].
With grid g_i = -1 + 2i/127, std 0.5:
    probs[r, i] = exp(-0.5*((g_i - m_r)/0.5)^2 + c0),  c0 = -ln(0.5) - 0.5*ln(2pi)
    dist[r, i]  = exp(probs[r, i]) / S_r,   S_r = sum_i exp(probs[r, i])
    alp[e, r]   = dist[r, action[e, r]]

Key simplifications:
  * The gather is recomputed directly from the action index a:
    z = s2*a + b_r (s2 = (2/127)/0.5, b_r = -2 - 2*m_r), so
    alp = exp(exp(-0.5*z^2 + c0)) * invS_r  -- pure elementwise work.
  * S_r is an analytic, even function of m_r alone; 1/S is evaluated by a
    hard-coded degree-8 polynomial in u = m^2 (max rel err ~1.6e-9), so the
    128-point grid sweep is never materialized.

Per core: rows [c*4096, (c+1)*4096). The host pre-permutes actions into the
SBUF-resident layout [partition p, group g, epoch e] (value = action for row
g*128+p, as bf16 exact small ints) so every large DMA is fully dense, and
un-permutes the [128, 4096] f32 outputs.
"""

import math
import numpy as np
import ml_dtypes

N_CORES = 8
ROWS = 32768
R = ROWS // N_CORES          # 4096 rows per core
E = 128                      # epochs
G = R // 128                 # 32 row-groups of 128
FEAT = 256
M_OUT = 512
LUTS_PER_CORE = 8
X_DIM = 128
THR_STD = 0.5
X_RANGE = 1.0

S2 = float((2.0 * X_RANGE / (X_DIM - 1)) / THR_STD)
C0 = float(-math.log(THR_STD) - 0.5 * math.log(2.0 * math.pi))

# 1/S(m) as an even polynomial in u = m^2 (degree 8, rel err ~1.6e-9),
# S(m) = sum_i exp(exp(-0.5*((g_i - m)/std)^2 + c0)) over the fixed grid.
INVS_COEF = [
    0.004734368204497658,
    0.0006664708139734746,
    0.0005536234448627335,
    -3.841141713816861e-05,
    -8.543615513324016e-05,
    -3.230374091879953e-05,
    3.099837949298949e-05,
    -2.056325921731732e-06,
    -1.460328775896884e-06,
]

CH = 16                      # row-groups per compute chunk ([128, CH*128] tiles)
SQ_ON_DVE = 0                # chunks (of 4) whose Square runs on VectorE

_CACHE = {}


def _col(d, g):
    # column of the per-row vectors (m, b, invS) holding group g of dist d.
    # Group g covers rows [g*128, (g+1)*128); row r = i*512 + jc*128 + p
    # (i = lut, jc = j-chunk, p = partition) => g = i*4 + jc.
    i, jc = g // 4, g % 4
    return d * 32 + jc * 8 + i


def _build():
    if "nc" in _CACHE:
        return _CACHE["nc"]

    from contextlib import ExitStack
    import concourse.bacc as bacc
    import concourse.mybir as mybir
    from concourse.tile import TileContext
    from concourse.alu_op_type import AluOpType

    dt = mybir.dt
    AF = mybir.ActivationFunctionType

    nc = bacc.Bacc("TRN2", target_bir_lowering=False, debug=False)

    a_ext = [
        nc.declare_dram_parameter("ax_t", [128, G * E], dt.bfloat16, isOutput=False),
        nc.declare_dram_parameter("ay_t", [128, G * E], dt.bfloat16, isOutput=False),
    ]
    w_ext = [
        nc.declare_dram_parameter("w_x", [FEAT, M_OUT], dt.float32, isOutput=False),
        nc.declare_dram_parameter("w_y", [FEAT, M_OUT], dt.float32, isOutput=False),
    ]
    b_ext = [
        nc.declare_dram_parameter("b_x", [128, 4], dt.float32, isOutput=False),
        nc.declare_dram_parameter("b_y", [128, 4], dt.float32, isOutput=False),
    ]
    xt_ext = nc.declare_dram_parameter(
        "x_t", [FEAT, LUTS_PER_CORE], dt.float32, isOutput=False
    )
    eye_ext = nc.declare_dram_parameter(
        "eye8", [LUTS_PER_CORE, LUTS_PER_CORE], dt.float32, isOutput=False
    )
    o_ext = [
        nc.declare_dram_parameter("out_x", [128, G * E], dt.float32, isOutput=True),
        nc.declare_dram_parameter("out_y", [128, G * E], dt.float32, isOutput=True),
    ]

    with TileContext(nc) as tc, ExitStack() as ctx:
        const = ctx.enter_context(tc.tile_pool(name="const", bufs=1))
        psum = ctx.enter_context(tc.tile_pool(name="psum", bufs=2, space="PSUM"))
        psum2p = ctx.enter_context(tc.tile_pool(name="psum2", bufs=2, space="PSUM"))
        zpool = ctx.enter_context(tc.tile_pool(name="zpool", bufs=2))
        z2pool = ctx.enter_context(tc.tile_pool(name="z2pool", bufs=2))
        ppool = ctx.enter_context(tc.tile_pool(name="ppool", bufs=2))
        epool = ctx.enter_context(tc.tile_pool(name="epool", bufs=2))
        opool = ctx.enter_context(tc.tile_pool(name="opool", bufs=3))

        # ---- constant / parameter loads -------------------------------
        w_sb = []
        b_sb = []
        for d in range(2):
            w = const.tile([128, 2, M_OUT], dt.float32, tag=f"w{d}")
            nc.sync.dma_start(
                out=w, in_=w_ext[d].ap().rearrange("(kc p) j -> p kc j", p=128)
            )
            w_sb.append(w)
            b = const.tile([128, 4], dt.float32, tag=f"b{d}")
            nc.sync.dma_start(out=b, in_=b_ext[d].ap())
            b_sb.append(b)
        xt_sb = const.tile([128, 2, LUTS_PER_CORE], dt.float32, name="xt")
        nc.sync.dma_start(
            out=xt_sb, in_=xt_ext.ap().rearrange("(kc p) i -> p kc i", p=128)
        )
        eye_sb = const.tile([LUTS_PER_CORE, LUTS_PER_CORE], dt.float32, name="eye8")
        nc.sync.dma_start(out=eye_sb, in_=eye_ext.ap())
        c0_bias = const.tile([128, 1], dt.float32, name="c0_bias")
        nc.vector.memset(c0_bias, C0)
        a_sb = []
        for d in range(2):
            a = const.tile([128, G * E], dt.bfloat16, tag=f"a{d}")
            nc.sync.dma_start(out=a, in_=a_ext[d].ap())
            a_sb.append(a)

        # ---- means ----------------------------------------------------
        # natural layout first: psum1[i, j] = (x @ W)[i, j] via 2 K-chunks,
        # then transpose 128-column chunks with an eye8 matmul and apply
        # tanh(. + b_j) with a per-partition bias.
        m_all = const.tile([128, 64], dt.float32, name="m_all")
        for d in range(2):
            pm = psum.tile([LUTS_PER_CORE, M_OUT], dt.float32)
            for kc in range(2):
                nc.tensor.matmul(
                    pm,
                    lhsT=xt_sb[:, kc, :],
                    rhs=w_sb[d][:, kc, :],
                    start=(kc == 0), stop=(kc == 1),
                )
            s_nat = const.tile([LUTS_PER_CORE, M_OUT], dt.float32, tag=f"s_nat{d}")
            nc.scalar.copy(s_nat, pm)
            for jc in range(4):
                pt = psum2p.tile([128, LUTS_PER_CORE], dt.float32)
                nc.tensor.matmul(
                    pt,
                    lhsT=s_nat[:, jc * 128:(jc + 1) * 128],
                    rhs=eye_sb,
                    start=True, stop=True,
                )
                nc.scalar.activation(
                    m_all[:, d * 32 + jc * 8: d * 32 + jc * 8 + 8],
                    pt, AF.Tanh, bias=b_sb[d][:, jc:jc + 1],
                )

        # ---- per-row prep: b_r = -2 - 2m, invS = poly(m^2) -------------
        u = const.tile([128, 64], dt.float32, name="u")
        nc.vector.tensor_tensor(u, m_all, m_all, AluOpType.mult)
        b_all = const.tile([128, 64], dt.float32, name="b_all")
        nc.vector.tensor_scalar(
            b_all, m_all, -2.0, -2.0, AluOpType.mult, AluOpType.add
        )
        h = const.tile([128, 64], dt.float32, name="h")
        t = const.tile([128, 64], dt.float32, name="t")
        nc.vector.tensor_scalar(
            h, u, float(INVS_COEF[8]), float(INVS_COEF[7]),
            AluOpType.mult, AluOpType.add,
        )
        for k in range(6, -1, -1):
            nc.vector.tensor_tensor(t, h, u, AluOpType.mult)
            nc.vector.tensor_scalar(h, t, float(INVS_COEF[k]), None, AluOpType.add)
        invs_all = h

        # ---- main loop: alp = exp(exp(-0.5 z^2 + c0)) * invS -----------
        n_chunks = G // CH
        chunk_idx = 0
        for d in range(2):
            for c2 in range(n_chunks):
                z = zpool.tile([128, CH, E], dt.float32)
                for gi in range(CH):
                    g = c2 * CH + gi
                    col = _col(d, g)
                    nc.vector.tensor_scalar(
                        z[:, gi, :],
                        a_sb[d][:, g * E:(g + 1) * E],
                        S2, b_all[:, col:col + 1],
                        AluOpType.mult, AluOpType.add,
                    )
                z2 = z2pool.tile([128, CH, E], dt.float32)
                if chunk_idx < SQ_ON_DVE:
                    for gi in range(CH):
                        nc.vector.tensor_tensor(
                            z2[:, gi, :], z[:, gi, :], z[:, gi, :], AluOpType.mult
                        )
                else:
                    nc.scalar.activation(z2, z, AF.Square)
                p = ppool.tile([128, CH, E], dt.float32)
                nc.scalar.activation(p, z2, AF.Exp, bias=c0_bias, scale=-0.5)
                e = epool.tile([128, CH, E], dt.float32)
                nc.scalar.activation(e, p, AF.Exp)
                o = opool.tile([128, CH, E], dt.float32)
                for gi in range(CH):
                    g = c2 * CH + gi
                    col = _col(d, g)
                    nc.vector.tensor_scalar(
                        o[:, gi, :], e[:, gi, :],
                        invs_all[:, col:col + 1], None,
                        AluOpType.mult,
                    )
                nc.sync.dma_start(
                    out=o_ext[d].ap()[:, c2 * CH * E:(c2 + 1) * CH * E], in_=o
                )
                chunk_idx += 1

    nc.compile()
    _CACHE["nc"] = nc
    return nc


LAST_RESULTS = None


def _to_device_layout(action):
    # [E, ROWS] int -> per-core [128, G*E] bf16 with value(p, g, e) =
    # action[e, core_base + g*128 + p]
    a = np.asarray(action).T.astype(np.float32).astype(ml_dtypes.bfloat16)
    a = a.reshape(N_CORES, G, 128, E)          # [core, g, p, e]
    a = a.transpose(0, 2, 1, 3)                # [core, p, g, e]
    return np.ascontiguousarray(a.reshape(N_CORES, 128, G * E))


def kernel(x, Wx, bx, Wy, by, action_x, action_y):
    global LAST_RESULTS
    from concourse.bass_utils import run_bass_kernel_spmd

    nc = _build()

    x = np.ascontiguousarray(np.asarray(x, dtype=np.float32))
    Wx = np.ascontiguousarray(np.asarray(Wx, dtype=np.float32))
    Wy = np.ascontiguousarray(np.asarray(Wy, dtype=np.float32))
    bx = np.ascontiguousarray(
        np.asarray(bx, dtype=np.float32).reshape(4, 128).T
    )
    by = np.ascontiguousarray(
        np.asarray(by, dtype=np.float32).reshape(4, 128).T
    )
    ax_d = _to_device_layout(action_x)
    ay_d = _to_device_layout(action_y)
    eye = np.ascontiguousarray(np.eye(LUTS_PER_CORE, dtype=np.float32))

    in_maps = []
    for c in range(N_CORES):
        in_maps.append({
            "ax_t": ax_d[c],
            "ay_t": ay_d[c],
            "w_x": Wx,
            "w_y": Wy,
            "b_x": bx,
            "b_y": by,
            "eye8": eye,
            "x_t": np.ascontiguousarray(
                x[c * LUTS_PER_CORE:(c + 1) * LUTS_PER_CORE].T
            ),
        })

    res = run_bass_kernel_spmd(nc, in_maps, core_ids=list(range(N_CORES)))
    LAST_RESULTS = res

    alp_x = np.empty((E, ROWS), dtype=np.float32)
    alp_y = np.empty((E, ROWS), dtype=np.float32)
    for c in range(N_CORES):
        for name, alp in (("out_x", alp_x), ("out_y", alp_y)):
            o = res.results[c][name].reshape(128, G, E)   # [p, g, e]
            o = o.transpose(2, 1, 0).reshape(E, R)        # [e, (g p)]
            alp[:, c * R:(c + 1) * R] = o
    ent = np.float32(0.5 + 0.5 * math.log(2.0 * math.pi) + math.log(THR_STD))
    return alp_x, alp_y, ent, ent


# revision 8
# speedup vs baseline: 1.3579x; 1.1467x over previous
"""Trainium2 Bass kernel for nn_Actor (sampling): per-row Gaussian-grid
softmax + per-row gather, data-parallel over 32768 distribution rows on 8
NeuronCores.

Math: for each row r (r = lut*512 + j), mean m_r = tanh(x @ W + b)[r].
With grid g_i = -1 + 2i/127, std 0.5:
    probs[r, i] = exp(-0.5*((g_i - m_r)/0.5)^2 + c0),  c0 = -ln(0.5) - 0.5*ln(2pi)
    dist[r, i]  = exp(probs[r, i]) / S_r,   S_r = sum_i exp(probs[r, i])
    alp[e, r]   = dist[r, action[e, r]]

Key simplifications:
  * The gather is recomputed directly from the action index a:
    with z = s2*a + b_r (s2 = (2/127)/0.5, b_r = -2 - 2*m_r):
    alp = exp(exp(-0.5*z^2 + c0)) * invS_r  -- pure elementwise work.
    On device: zd = a + b_r/s2 (one tensor_tensor add against a 0-stride
    broadcast of the per-row vector), then Square applies the s2 scale.
  * S_r is an analytic, even function of m_r alone; 1/S is evaluated by a
    hard-coded degree-8 polynomial in u = m^2 (max rel err ~1.6e-9), so the
    128-point grid sweep is never materialized.

Per core: rows [c*4096, (c+1)*4096). The host pre-permutes actions into the
SBUF-resident layout [partition p, group g, epoch e] (value = action for row
g*128+p, as bf16 exact small ints) so every large DMA is fully dense, and
un-permutes the [128, 4096] f32 outputs.
"""

import math
import numpy as np
import ml_dtypes

N_CORES = 8
ROWS = 32768
R = ROWS // N_CORES          # 4096 rows per core
E = 128                      # epochs
G = R // 128                 # 32 row-groups of 128
FEAT = 256
M_OUT = 512
LUTS_PER_CORE = 8
X_DIM = 128
THR_STD = 0.5
X_RANGE = 1.0

S2 = float((2.0 * X_RANGE / (X_DIM - 1)) / THR_STD)
C0 = float(-math.log(THR_STD) - 0.5 * math.log(2.0 * math.pi))

# 1/S(m) as an even polynomial in u = m^2 (degree 8, rel err ~1.6e-9),
# S(m) = sum_i exp(exp(-0.5*((g_i - m)/std)^2 + c0)) over the fixed grid.
INVS_COEF = [
    0.004734368204497658,
    0.0006664708139734746,
    0.0005536234448627335,
    -3.841141713816861e-05,
    -8.543615513324016e-05,
    -3.230374091879953e-05,
    3.099837949298949e-05,
    -2.056325921731732e-06,
    -1.460328775896884e-06,
]

CH = 16                      # row-groups per compute chunk ([128, CH*128] tiles)
SQ_ON_DVE = (2,)             # chunk indices (0..3) whose Square runs on VectorE

_CACHE = {}


def _build():
    if "nc" in _CACHE:
        return _CACHE["nc"]

    from contextlib import ExitStack
    import concourse.bacc as bacc
    import concourse.mybir as mybir
    from concourse.tile import TileContext
    from concourse.alu_op_type import AluOpType

    dt = mybir.dt
    AF = mybir.ActivationFunctionType

    nc = bacc.Bacc("TRN2", target_bir_lowering=False, debug=False)

    a_ext = [
        nc.declare_dram_parameter("ax_t", [128, G * E], dt.bfloat16, isOutput=False),
        nc.declare_dram_parameter("ay_t", [128, G * E], dt.bfloat16, isOutput=False),
    ]
    w_ext = [
        nc.declare_dram_parameter("w_x", [FEAT, M_OUT], dt.float32, isOutput=False),
        nc.declare_dram_parameter("w_y", [FEAT, M_OUT], dt.float32, isOutput=False),
    ]
    b_ext = [
        nc.declare_dram_parameter("b_x", [128, 4], dt.float32, isOutput=False),
        nc.declare_dram_parameter("b_y", [128, 4], dt.float32, isOutput=False),
    ]
    xt_ext = nc.declare_dram_parameter(
        "x_t", [FEAT, LUTS_PER_CORE], dt.float32, isOutput=False
    )
    eye_ext = nc.declare_dram_parameter(
        "eye8", [LUTS_PER_CORE, LUTS_PER_CORE], dt.float32, isOutput=False
    )
    o_ext = [
        nc.declare_dram_parameter("out_x", [128, G * E], dt.float32, isOutput=True),
        nc.declare_dram_parameter("out_y", [128, G * E], dt.float32, isOutput=True),
    ]

    with TileContext(nc) as tc, ExitStack() as ctx:
        const = ctx.enter_context(tc.tile_pool(name="const", bufs=1))
        psum = ctx.enter_context(tc.tile_pool(name="psum", bufs=2, space="PSUM"))
        psum2p = ctx.enter_context(tc.tile_pool(name="psum2", bufs=2, space="PSUM"))
        zpool = ctx.enter_context(tc.tile_pool(name="zpool", bufs=2))
        z2pool = ctx.enter_context(tc.tile_pool(name="z2pool", bufs=2))
        ppool = ctx.enter_context(tc.tile_pool(name="ppool", bufs=2))
        epool = ctx.enter_context(tc.tile_pool(name="epool", bufs=2))
        opool = ctx.enter_context(tc.tile_pool(name="opool", bufs=3))

        # ---- constant / parameter loads -------------------------------
        w_sb, b_sb, a_sb = [], [], []
        for d in range(2):
            w = const.tile([128, 2, M_OUT], dt.float32, tag=f"w{d}")
            nc.sync.dma_start(
                out=w, in_=w_ext[d].ap().rearrange("(kc p) j -> p kc j", p=128)
            )
            w_sb.append(w)
            b = const.tile([128, 4], dt.float32, tag=f"b{d}")
            nc.sync.dma_start(out=b, in_=b_ext[d].ap())
            b_sb.append(b)
        xt_sb = const.tile([128, 2, LUTS_PER_CORE], dt.float32, name="xt")
        nc.sync.dma_start(
            out=xt_sb, in_=xt_ext.ap().rearrange("(kc p) i -> p kc i", p=128)
        )
        eye_sb = const.tile([LUTS_PER_CORE, LUTS_PER_CORE], dt.float32, name="eye8")
        nc.sync.dma_start(out=eye_sb, in_=eye_ext.ap())
        c0_bias = const.tile([128, 1], dt.float32, name="c0_bias")
        nc.vector.memset(c0_bias, C0)
        for d in range(2):
            a = const.tile([128, G * E], dt.bfloat16, tag=f"a{d}")
            nc.sync.dma_start(out=a, in_=a_ext[d].ap())
            a_sb.append(a)

        # ---- per-dist: means, per-row prep, main loop -----------------
        # means: psum1[i, j] = (x @ W)[i, j] via 2 K-chunks, then transpose
        # 128-column chunks with an eye8 matmul; tanh(. + b_j) lands the
        # strided write so that m_d[:, g] is the mean column of row-group g.
        def means(d):
            m_d = const.tile([128, G], dt.float32, tag=f"m{d}")
            pm = psum.tile([LUTS_PER_CORE, M_OUT], dt.float32)
            for kc in range(2):
                nc.tensor.matmul(
                    pm,
                    lhsT=xt_sb[:, kc, :],
                    rhs=w_sb[d][:, kc, :],
                    start=(kc == 0), stop=(kc == 1),
                )
            s_nat = const.tile([LUTS_PER_CORE, M_OUT], dt.float32, tag=f"s_nat{d}")
            nc.scalar.copy(s_nat, pm)
            for jc in range(4):
                pt = psum2p.tile([128, LUTS_PER_CORE], dt.float32)
                nc.tensor.matmul(
                    pt,
                    lhsT=s_nat[:, jc * 128:(jc + 1) * 128],
                    rhs=eye_sb,
                    start=True, stop=True,
                )
                # group g = i*4 + jc  ->  m_d[:, jc::4]
                nc.scalar.activation(
                    m_d[:, jc:G:4], pt, AF.Tanh, bias=b_sb[d][:, jc:jc + 1],
                )
            return m_d

        def prep(d, m_d):
            # bdiv = b_r / s2 = (-2 - 2 m)/s2 ; invS = poly(m^2)
            u = const.tile([128, G], dt.float32, tag=f"u{d}")
            nc.vector.tensor_tensor(u, m_d, m_d, AluOpType.mult)
            bdiv = const.tile([128, G], dt.float32, tag=f"bdiv{d}")
            nc.vector.tensor_scalar(
                bdiv, m_d, -2.0 / S2, -2.0 / S2, AluOpType.mult, AluOpType.add
            )
            h = const.tile([128, G], dt.float32, tag=f"h{d}")
            t = const.tile([128, G], dt.float32, tag=f"t{d}")
            nc.vector.tensor_scalar(
                h, u, float(INVS_COEF[8]), float(INVS_COEF[7]),
                AluOpType.mult, AluOpType.add,
            )
            for k in range(6, -1, -1):
                nc.vector.tensor_tensor(t, h, u, AluOpType.mult)
                nc.vector.tensor_scalar(
                    h, t, float(INVS_COEF[k]), None, AluOpType.add
                )
            return bdiv, h

        def main_loop(d, bdiv, invs):
            a_re = a_sb[d][:, :].rearrange("p (g e) -> p g e", e=E)
            n_chunks = G // CH
            for c2 in range(n_chunks):
                g0 = c2 * CH
                bd_b = bdiv[:, g0:g0 + CH].to_broadcast([128, CH, E])
                iv_b = invs[:, g0:g0 + CH].to_broadcast([128, CH, E])
                z = zpool.tile([128, CH, E], dt.float32)
                nc.vector.tensor_tensor(
                    z, a_re[:, g0:g0 + CH, :], bd_b, AluOpType.add
                )
                z2 = z2pool.tile([128, CH, E], dt.float32)
                if c2 + 2 * d in SQ_ON_DVE:
                    nc.vector.tensor_tensor(z2, z, z, AluOpType.mult)
                    exp_scale = -0.5 * S2 * S2
                else:
                    nc.scalar.activation(z2, z, AF.Square, scale=S2)
                    exp_scale = -0.5
                p = ppool.tile([128, CH, E], dt.float32)
                nc.scalar.activation(p, z2, AF.Exp, bias=c0_bias, scale=exp_scale)
                e = epool.tile([128, CH, E], dt.float32)
                nc.scalar.activation(e, p, AF.Exp)
                o = opool.tile([128, CH, E], dt.float32)
                nc.vector.tensor_tensor(o, e, iv_b, AluOpType.mult)
                nc.sync.dma_start(
                    out=o_ext[d].ap()[:, g0 * E:(g0 + CH) * E], in_=o
                )

        m0 = means(0)
        bdiv0, invs0 = prep(0, m0)
        m1 = means(1)
        bdiv1, invs1 = prep(1, m1)
        main_loop(0, bdiv0, invs0)
        main_loop(1, bdiv1, invs1)

    nc.compile()
    _CACHE["nc"] = nc
    return nc


LAST_RESULTS = None


def _to_device_layout(action):
    # [E, ROWS] int -> per-core [128, G*E] bf16 with value(p, g, e) =
    # action[e, core_base + g*128 + p]
    a = np.asarray(action).T.astype(np.float32).astype(ml_dtypes.bfloat16)
    a = a.reshape(N_CORES, G, 128, E)          # [core, g, p, e]
    a = a.transpose(0, 2, 1, 3)                # [core, p, g, e]
    return np.ascontiguousarray(a.reshape(N_CORES, 128, G * E))


def kernel(x, Wx, bx, Wy, by, action_x, action_y):
    global LAST_RESULTS
    from concourse.bass_utils import run_bass_kernel_spmd

    nc = _build()

    x = np.ascontiguousarray(np.asarray(x, dtype=np.float32))
    Wx = np.ascontiguousarray(np.asarray(Wx, dtype=np.float32))
    Wy = np.ascontiguousarray(np.asarray(Wy, dtype=np.float32))
    bx = np.ascontiguousarray(np.asarray(bx, dtype=np.float32).reshape(4, 128).T)
    by = np.ascontiguousarray(np.asarray(by, dtype=np.float32).reshape(4, 128).T)
    ax_d = _to_device_layout(action_x)
    ay_d = _to_device_layout(action_y)
    eye = np.ascontiguousarray(np.eye(LUTS_PER_CORE, dtype=np.float32))

    in_maps = []
    for c in range(N_CORES):
        in_maps.append({
            "ax_t": ax_d[c],
            "ay_t": ay_d[c],
            "w_x": Wx,
            "w_y": Wy,
            "b_x": bx,
            "b_y": by,
            "eye8": eye,
            "x_t": np.ascontiguousarray(
                x[c * LUTS_PER_CORE:(c + 1) * LUTS_PER_CORE].T
            ),
        })

    res = run_bass_kernel_spmd(nc, in_maps, core_ids=list(range(N_CORES)))
    LAST_RESULTS = res

    alp_x = np.empty((E, ROWS), dtype=np.float32)
    alp_y = np.empty((E, ROWS), dtype=np.float32)
    for c in range(N_CORES):
        for name, alp in (("out_x", alp_x), ("out_y", alp_y)):
            o = res.results[c][name].reshape(128, G, E)   # [p, g, e]
            o = o.transpose(2, 1, 0).reshape(E, R)        # [e, (g p)]
            alp[:, c * R:(c + 1) * R] = o
    ent = np.float32(0.5 + 0.5 * math.log(2.0 * math.pi) + math.log(THR_STD))
    return alp_x, alp_y, ent, ent
